# revision 2
# baseline (speedup 1.0000x reference)
"""DecoderAttentionGRU Trainium2 kernel, v2: two-group software pipeline.

Per-core problem (batch shard Bs=16 of B=128 over 8 cores). The 16
sequences split into two independent groups of G=8, advanced in
alternating "slots": while ACT runs group X's big z=tanh(hp+sp) call
(~7.1us, the per-step floor), PE/DVE run group Y's attention tail
(e-dot, softmax, context, GRU) and prepare Y's next zin. This fills the
ACT idle gaps that serialize the single-recurrence baseline.

Slot(X, t) emission = [tail_pre(Y, tY): y-head GEMMs+tanh, e-dot, exp]
                      [z_X(t): one ACT call on [128, HC*G*T]=8192 elems]
                      [tail_post(Y, tY): softmax, aT, c-dot, GRU, s/sp
                       update, zin(t+1) adds, y output]
so the ACT FIFO is [y1t_Y, y2t_Y, exp_Y, z_X, tr/tu/hcand_Y] - the only
ACT ops ahead of z_X are ready early in the slot.

e-dot: out row 32j (col-group j via tile_position) accumulates over c
the [1,512] strip for batch pair (2j, 2j+1); free = (k=b%2, t).
sigmoid avoided as in v1: sig(v)=0.5*(1+tanh(v/2)); Whh pre-scaled 0.5.
"""

import numpy as np
from contextlib import ExitStack

import concourse.bass as bass
import concourse.tile as tile
from concourse import bacc, mybir
from concourse.bass import ds, ts
from concourse.masks import make_identity

F16 = mybir.dt.float16
F32 = mybir.dt.float32
F8 = mybir.dt.float8e4
AF = mybir.ActivationFunctionType
ALU = mybir.AluOpType
AX = mybir.AxisListType

P = 128
B, T, H, O = 128, 256, 512, 256
X = O + H            # 768
NCORES = 8
Bs = B // NCORES     # 16
G = 8                # group size (2 groups per core)
HC, TC, OC, XC = H // P, T // P, O // P, X // P   # 4, 2, 2, 6


FP8_WEIGHTS = ("Wxr", "Wxu", "Whr", "Whu", "We1s")


def build_nc(nsteps=T, kf=5, dynamic=True, bias_on=None, fp8=False):
    if bias_on is None:
        bias_on = {k: False for k in ("by1", "by2", "by3", "be1", "br", "bu")}
    assert (nsteps - 1) % kf == 0, "loop covers t=1..nsteps-1"

    nc = bacc.Bacc("TRN2", target_bir_lowering=False, debug=False)

    h_d = nc.dram_tensor("h", [Bs, T, H], F32, kind="ExternalInput")
    s0_d = nc.dram_tensor("s0", [Bs, H], F32, kind="ExternalInput")
    wd = {}
    for name, shape in [
        ("Wy1", [H, H]), ("by1", [H]), ("Wy2", [H, H]), ("by2", [H]),
        ("Wy3", [H, O]), ("by3", [O]), ("We1", [2 * H, H]), ("be1", [H]),
        ("We2", [H, 1]), ("be2", [1]),
        ("Wxr", [X, H]), ("Whr", [H, H]), ("br", [H]),
        ("Wxu", [X, H]), ("Whu", [H, H]), ("bu", [H]),
        ("Wxh", [X, H]), ("Whh", [H, H]),
    ]:
        wd[name] = nc.dram_tensor(name, shape, F32, kind="ExternalInput")
    out_d = nc.dram_tensor("out", [Bs, T, O], F32, kind="ExternalOutput")

    with tile.TileContext(nc) as tc, ExitStack() as top:
        # ---------------- persistent SBUF ----------------
        pconst = top.enter_context(tc.tile_pool(name="const", bufs=1))
        pstate = top.enter_context(tc.tile_pool(name="state", bufs=1))

        ident16 = pconst.tile([P, P], F16, tag="id16", name="id16")
        identf32 = pconst.tile([P, P], F32, tag="id32", name="id32")
        make_identity(nc, ident16)
        make_identity(nc, identf32)
        ones16 = pconst.tile([1, G], F16, tag="ones", name="ones")
        nc.vector.memset(ones16, 1.0)
        onescol = pconst.tile([P, 1], F16, tag="onescol", name="onescol")
        nc.vector.memset(onescol, 1.0)
        onesrow = pconst.tile([1, P], F32, tag="onesrow", name="onesrow")
        nc.vector.memset(onesrow, 1.0)

        def wdt(name):
            return F8 if (fp8 and name in FP8_WEIGHTS) else F16

        w16 = {}
        for name, kc, m in [("Wy1", HC, H), ("Wy2", HC, H), ("Wy3", HC, O),
                            ("Wxr", XC, H), ("Wxu", XC, H), ("Wxh", XC, H),
                            ("Whr", HC, H), ("Whu", HC, H), ("Whh", HC, H)]:
            w16[name] = pconst.tile([P, kc, m], wdt(name), tag=f"w_{name}", name=f"w_{name}")
        w16["We1s"] = pconst.tile([P, HC, H], wdt("We1s"), tag="w_We1s", name="w_We1s")
        we2_16 = pconst.tile([P, HC], F16, tag="w_We2", name="w_We2")
        b16 = {}
        for name, m in [("by1", H), ("by2", H), ("by3", O), ("be1", H),
                        ("br", H), ("bu", H)]:
            if bias_on[name]:
                b16[name] = pconst.tile([1, m], F16, tag=f"b_{name}", name=f"b_{name}")

        h16 = pstate.tile([P, Bs, TC, H], F16, tag="h16", name="h16")   # [t%128,(b,tc,h)]
        hp = pstate.tile([P, HC, Bs, T], F16, tag="hp", name="hp")      # [h'%128,(c,b,t)]
        # per-group state
        s_f32 = [pstate.tile([P, HC * G], F32, tag=f"s{g}", name=f"s{g}")
                 for g in range(2)]
        sp_f = [pstate.tile([P, HC * G], F32, tag=f"spf{g}", name=f"spf{g}")
                for g in range(2)]
        zin = [pstate.tile([P, HC, G, T], F16, tag=f"zin{g}", name=f"zin{g}")
               for g in range(2)]
        # shared tanh output: consumed (e-dot) within the same z-slot, so
        # both groups can share one buffer; avoids in-place ACT read/write
        zsh = pstate.tile([P, HC, G, T], F16, tag="zsh", name="zsh")
        p_sbs = [pstate.tile([P, 2, T], F16, tag=f"psb_{g}", name=f"psb_{g}")
                 for g in range(2)]
        esums = [pstate.tile([P, 2], F32, tag=f"esum_{g}", name=f"esum_{g}")
                 for g in range(2)]

        # ---------------- prologue: weights ----------------
        with tc.tile_pool(name="stage", bufs=2) as pstg:

            def load_w16(dst, dram_ap, kc, m, scale=None):
                stg = pstg.tile([P, kc, m], F32, tag="wstage", name="wstage")
                nc.sync.dma_start(
                    stg[:, :kc, :m],
                    dram_ap.rearrange("(kc p) m -> p kc m", p=P))
                src = stg[:, :kc, :m].rearrange("p kc m -> p (kc m)")
                dstv = dst.rearrange("p kc m -> p (kc m)")
                if scale is None:
                    nc.vector.tensor_copy(dstv, src)
                else:
                    nc.vector.tensor_scalar_mul(dstv, src, scale)

            for name in ("Wy1", "Wy2", "Wxr", "Wxu", "Wxh", "Whr", "Whu"):
                d = wd[name]
                load_w16(w16[name], d[:, :], d.shape[0] // P, d.shape[1])
            load_w16(w16["Whh"], wd["Whh"][:, :], HC, H)
            load_w16(w16["Wy3"], wd["Wy3"][:, :], HC, O)
            load_w16(w16["We1s"], wd["We1"][H:, :], HC, H)
            stg = pstg.tile([P, HC], F32, tag="we2stage", name="we2stage")
            nc.sync.dma_start(stg[:], wd["We2"][:, 0].rearrange("(c p) -> p c", p=P))
            nc.vector.tensor_copy(we2_16[:], stg[:])
            for name in b16:
                m = b16[name].shape[1]
                stg = pstg.tile([1, m], F32, tag="bstage", name="bstage")
                nc.sync.dma_start(stg[:1, :m], wd[name][None, :])
                nc.vector.tensor_copy(b16[name][:], stg[:1, :m])

        # ---------------- prologue: h, s0, h_proj ----------------
        with tc.tile_pool(name="stage2", bufs=2) as pstg, \
             tc.tile_pool(name="pps", bufs=2, space="PSUM") as pps, \
             tc.tile_pool(name="hTb", bufs=1) as phT:

            for bb in range(Bs):
                h32 = pstg.tile([P, TC, H], F32, tag="h32", name="h32")
                nc.sync.dma_start(
                    h32[:], h_d[bb, :, :].rearrange(
                        "(tc p) hh -> p tc hh", p=P))
                nc.vector.tensor_copy(
                    h16[:, bb].rearrange("p tc hh -> p (tc hh)"),
                    h32.rearrange("p tc hh -> p (tc hh)"))

            s0stg = pstg.tile([Bs, H], F32, tag="s0stage", name="s0stage")
            nc.sync.dma_start(s0stg[:], s0_d[:, :])
            for c in range(HC):
                ps = pps.tile([P, Bs], F32, tag="s0ps", name="s0ps")
                nc.tensor.transpose(ps[:], s0stg[:, c * P:(c + 1) * P],
                                    identf32[:Bs, :Bs])
                for g in range(2):
                    nc.vector.tensor_copy(
                        s_f32[g][:, c * G:(c + 1) * G], ps[:, g * G:(g + 1) * G])

            w1h = pstg.tile([P, HC, H], F16, tag="w1hstage", name="w1hstage")
            stg = pstg.tile([P, HC, H], F32, tag="w1hstg32", name="w1hstg32")
            nc.sync.dma_start(stg[:], wd["We1"][:H, :].rearrange(
                "(kc p) m -> p kc m", p=P))
            nc.vector.tensor_copy(
                w1h.rearrange("p kc m -> p (kc m)"),
                stg.rearrange("p kc m -> p (kc m)"))

            for bb in range(0, Bs, 4):
                hT = phT.tile([P, HC, 4, T], F16, tag="hT", name="hT")
                for b4 in range(4):
                    for tcc in range(TC):
                        for c in range(HC):
                            ps = pps.tile([P, P], F16, tag="hTps", name="hTps")
                            nc.tensor.transpose(
                                ps[:], h16[:, bb + b4, tcc, c * P:(c + 1) * P],
                                ident16)
                            nc.vector.tensor_copy(
                                hT[:, c, b4, tcc * P:(tcc + 1) * P], ps[:])
                hTv = hT.rearrange("p c b t -> p c (b t)")
                for m in range(HC):
                    for nb in range(2):
                        ps = pps.tile([P, 512], F32, tag="hpps", name="hpps")
                        for k in range(HC):
                            nc.tensor.matmul(
                                ps[:], w1h[:, k, m * P:(m + 1) * P],
                                hTv[:, k, nb * 512:(nb + 1) * 512],
                                start=(k == 0), stop=(k == HC - 1))
                        nc.vector.tensor_copy(
                            hp[:, m, bb:bb + 4].rearrange("p b t -> p (b t)")[
                                :, nb * 512:(nb + 1) * 512], ps[:])

        # ---------------- steady-state pools ----------------
        psm = top.enter_context(tc.tile_pool(name="small", bufs=3))
        pyb = top.enter_context(tc.tile_pool(name="ybuf", bufs=2))
        ppe = top.enter_context(tc.tile_pool(name="pe_e", bufs=2, space="PSUM"))
        pg32 = top.enter_context(tc.tile_pool(name="pg32", bufs=4, space="PSUM"))

        pyT = top.enter_context(tc.tile_pool(name="pyT", bufs=1, space="PSUM"))
        paT = top.enter_context(tc.tile_pool(name="paT", bufs=1, space="PSUM"))

        out_flat = out_d[:, :, :].rearrange("b t o -> b (t o)")

        def mm_part(out_ps, mcount, terms, first, last):
            """terms: list of (w, k_indices, rhs); accumulate into out_ps."""
            for m in range(mcount):
                seq = []
                for (w, ks, rhs) in terms:
                    for k in ks:
                        seq.append((w[:, k, m * P:(m + 1) * P],
                                    rhs[:, k * G:(k + 1) * G]))
                n = len(seq)
                for i, (lhsT, rhs) in enumerate(seq):
                    nc.tensor.matmul(out_ps[:, m * G:(m + 1) * G], lhsT, rhs,
                                     start=(first and i == 0),
                                     stop=(last and i == n - 1))

        def mm_block(out_ps, mcount, terms, bias=None):
            """out_ps[:, m*G:(m+1)*G] += sum_terms W[k,m].T @ rhs[k] (+bias)"""
            for m in range(mcount):
                seq = []
                for (w, kc, rhs) in terms:
                    for k in range(kc):
                        seq.append((w[:, k, m * P:(m + 1) * P],
                                    rhs[:, k * G:(k + 1) * G]))
                if bias is not None:
                    seq.append((bias[:1, m * P:(m + 1) * P], ones16[:1, :]))
                n = len(seq)
                for i, (lhsT, rhs) in enumerate(seq):
                    nc.tensor.matmul(out_ps[:, m * G:(m + 1) * G], lhsT, rhs,
                                     start=(i == 0), stop=(i == n - 1))

        # exp/tanh share one ACT table set; e_ps garbage rows stay finite
        # because each e_ps buffer is memset once here.
        e_tiles = [ppe.tile([P, 2, T], F32, tag="eps", name="eps")
                   for _ in range(2)]
        for e_t in e_tiles:
            nc.vector.memset(e_t.rearrange("p k t -> p (k t)"), 0.0)

        # per-(group, phase) scratch passed pre->post
        ctx = [dict(), dict()]

        def high_prio_range(n):
            # context helper: emit the following matmuls at priority 0
            with tc.high_priority():
                for j in range(n):
                    yield j

        def emit_z(g, c=None):
            if c is None:
                for ci in range(HC):
                    emit_z(g, ci)
                emit_exp(g)
                return
            vin = zin[g][:, c].rearrange("p b t -> p (b t)")
            vout = zsh[:, c].rearrange("p b t -> p (b t)")
            nc.scalar.activation(vout, vin, AF.Tanh)
            e_ps = e_tiles[g]
            with tc.high_priority():
                for j in range(4):
                    nc.tensor.matmul(
                        e_ps[32 * j:32 * j + 1, :, :].rearrange("q k t -> q (k t)"),
                        we2_16[:, c:c + 1],
                        zsh[:, c, 2 * j:2 * j + 2, :].rearrange("p b t -> p (b t)"),
                        start=(c == 0), stop=(c == HC - 1),
                        tile_position=(0, 32 * j))

        def emit_exp(g):
            p_sb, esum = p_sbs[g], esums[g]
            with tc.high_priority():
                for k in range(2):
                    nc.scalar.activation(p_sb[:, k, :], e_tiles[g][:, k, :],
                                         AF.Exp, accum_out=esum[:, k:k + 1])

        def tail_pre(g, t):
            cx = ctx[g]
            sT16 = psm.tile([P, HC * G], F16, tag="sT16", name="sT16")
            nc.vector.tensor_copy(sT16[:], s_f32[g][:])
            cx["sT16"] = sT16

            y1ps = ppy.tile([P, HC * G], F32, tag="yps", name="yps")
            mm_block(y1ps, HC, [(w16["Wy1"], HC, sT16)], b16.get("by1"))
            y1 = psm.tile([P, HC * G], F16, tag="y1", name="y1")
            nc.scalar.activation(y1[:], y1ps[:], AF.Tanh)
            y2ps = ppy.tile([P, HC * G], F32, tag="yps", name="yps")
            mm_block(y2ps, HC, [(w16["Wy2"], HC, y1)], b16.get("by2"))
            y2 = psm.tile([P, HC * G], F16, tag="y2", name="y2")
            nc.scalar.activation(y2[:], y2ps[:], AF.Tanh)
            y3ps = ppy.tile([P, OC * G], F32, tag="yps", name="yps")
            mm_block(y3ps, OC, [(w16["Wy3"], HC, y2)], b16.get("by3"))
            yT3 = psm.tile([P, OC * G], F32, tag="yT3", name="yT3")
            nc.vector.tensor_copy(yT3[:], y3ps[:])
            cx["yT3"] = yT3
            xT = psm.tile([P, XC * G], F16, tag="xT", name="xT")
            nc.vector.tensor_copy(xT[:, :OC * G], y3ps[:])
            cx["xT"] = xT


        def post_seg1(g):
            cx = ctx[g]
            sT16 = cx["sT16"]
            p_sb, esum = p_sbs[g], esums[g]
            rcp = psm.tile([P, 2], F32, tag="rcp", name="rcp")
            a_sb = psm.tile([P, 2, T], F16, tag="asb", name="asb")
            aT_ps = paT.tile([P, TC, 2, 4], F32, tag="aTps", name="aTps")
            aT16 = psm.tile([P, TC, G], F16, tag="aT16", name="aT16")
            with tc.high_priority():
                nc.vector.reciprocal(rcp[:], esum[:])
                for k in range(2):
                    nc.vector.tensor_scalar_mul(a_sb[:, k, :], p_sb[:, k, :],
                                                rcp[:, k:k + 1])
                for tc_i in range(TC):
                    for k in range(2):
                        nc.tensor.matmul(
                            aT_ps[:, tc_i, k, :],
                            a_sb[:, k, tc_i * P:(tc_i + 1) * P],
                            ident16[:, 0:P:32], start=True, stop=True)
                nc.vector.tensor_copy(
                    aT16.rearrange("p tc (j k) -> p tc k j", k=2),
                    aT_ps[:, :, :, :])
            cT_ps = pg32.tile([P, HC * G], F32, tag="ps32", name="ps32")
            for b in range(G):
                bg = g * G + b
                for c2 in range(HC):
                    for tcc in range(TC):
                        nc.tensor.matmul(
                            cT_ps[:, c2 * G + b:c2 * G + b + 1],
                            h16[:, bg, tcc, c2 * P:(c2 + 1) * P],
                            aT16[:, tcc, b:b + 1],
                            start=(tcc == 0), stop=(tcc == TC - 1))
            xT = cx["xT"]
            with tc.high_priority():
                nc.vector.tensor_copy(xT[:, OC * G:], cT_ps[:])
            rps = pg32.tile([P, HC * G], F32, tag="ps32", name="ps32")
            mm_block(rps, HC, [(w16["Wxr"], XC, xT), (w16["Whr"], HC, sT16)],
                     b16.get("br"))
            ups = pg32.tile([P, HC * G], F32, tag="ps32", name="ps32")
            mm_block(ups, HC, [(w16["Wxu"], XC, xT), (w16["Whu"], HC, sT16)],
                     b16.get("bu"))
            cx["rps"], cx["ups"] = rps, ups

        def post_seg2(g):
            cx = ctx[g]
            tr = psm.tile([P, HC * G], F32, tag="tr", name="tr")
            nc.scalar.activation(tr[:], cx["rps"][:], AF.Tanh, scale=0.5)
            r16 = psm.tile([P, HC * G], F32, tag="r16", name="r16")
            rs16 = psm.tile([P, HC * G], F16, tag="rs16", name="rs16")
            with tc.high_priority():
                nc.vector.tensor_scalar(r16[:], tr[:], 0.5, 0.5,
                                        ALU.mult, ALU.add)
                nc.vector.tensor_tensor(rs16[:], r16[:], s_f32[g][:], ALU.mult)
            hps = pg32.tile([P, HC * G], F32, tag="ps32", name="ps32")
            mm_block(hps, HC, [(w16["Wxh"], XC, cx["xT"]),
                               (w16["Whh"], HC, rs16)])
            cx["hps"] = hps

        def post_seg3(g, last=False):
            cx = ctx[g]
            tu = psm.tile([P, HC * G], F32, tag="tu", name="tu")
            nc.scalar.activation(tu[:], cx["ups"][:], AF.Tanh, scale=0.5)
            hcand = psm.tile([P, HC * G], F32, tag="hcand", name="hcand")
            nc.scalar.activation(hcand[:], cx["hps"][:], AF.Tanh)
            d_ = psm.tile([P, HC * G], F32, tag="d_", name="d_")
            q_ = psm.tile([P, HC * G], F32, tag="q_", name="q_")
            with tc.high_priority():
                nc.vector.tensor_tensor(d_[:], s_f32[g][:], hcand[:], ALU.subtract)
                nc.vector.tensor_tensor(q_[:], tu[:], d_[:], ALU.mult)
                nc.vector.tensor_tensor(q_[:], q_[:], d_[:], ALU.add)
                nc.vector.tensor_scalar_mul(q_[:], q_[:], 0.5)
                nc.vector.tensor_tensor(s_f32[g][:], hcand[:], q_[:], ALU.add)
            if not last:
                sT16b = psm.tile([P, HC * G], F16, tag="sT16b", name="sT16b")
                sp_ps = pg32.tile([P, HC * G], F32, tag="ps32", name="ps32")
                with tc.high_priority():
                    nc.vector.tensor_copy(sT16b[:], s_f32[g][:])
                mm_block(sp_ps, HC, [(w16["We1s"], HC, sT16b)], b16.get("be1"))
                with tc.high_priority():
                    nc.vector.tensor_copy(sp_f[g][:], sp_ps[:])
                    for b in range(G):
                        nc.vector.tensor_scalar_add(
                            zin[g][:, 0, b, :], hp[:, 0, g * G + b, :],
                            sp_f[g][:, b:b + 1])

        def post_seg4(g, ksub, y_buf, last=False):
            cx = ctx[g]
            if not last:
                for c in range(1, HC):
                    eng = nc.vector
                    for b in range(G):
                        eng.tensor_scalar_add(
                            zin[g][:, c, b, :], hp[:, c, g * G + b, :],
                            sp_f[g][:, c * G + b:c * G + b + 1])
            yT_ps = pyT.tile([G, O], F32, tag="yTps", name="yTps")
            for oc in range(OC):
                nc.tensor.transpose(yT_ps[:, oc * P:(oc + 1) * P],
                                    cx["yT3"][:, oc * G:(oc + 1) * G], identf32)
            nc.vector.tensor_copy(y_buf[:, ksub, :], yT_ps[:])

        def emit_slot(gz, gt, ksub, y_buf, do_z=True, do_tail=True, last=False):
            """ACT FIFO: [y1,y2,exp(gt), zc0,zc1(gz), tr(gt), zc2, tu,hcand(gt), zc3]"""
            if do_tail:
                tail_pre(gt, None)
                post_seg1(gt)
            if do_z:
                emit_z(gz, 0)
                emit_z(gz, 1)
            if do_tail:
                post_seg2(gt)
            if do_z:
                emit_z(gz, 2)
            if do_tail:
                post_seg3(gt, last=last)
            if do_z:
                emit_z(gz, 3)
                emit_exp(gz)
            if do_tail:
                post_seg4(gt, ksub, y_buf, last=last)

        def prime_group(g):
            sT16b = psm.tile([P, HC * G], F16, tag="sT16b", name="sT16b")
            nc.vector.tensor_copy(sT16b[:], s_f32[g][:])
            sp_ps = ppx.tile([P, HC * G], F32, tag="spps", name="spps")
            mm_block(sp_ps, HC, [(w16["We1s"], HC, sT16b)], b16.get("be1"))
            nc.vector.tensor_copy(sp_f[g][:], sp_ps[:])
            for c in range(HC):
                for b in range(G):
                    nc.vector.tensor_scalar_add(
                        zin[g][:, c, b, :], hp[:, c, g * G + b, :],
                        sp_f[g][:, c * G + b:c * G + b + 1])

        # ---------------- schedule ----------------
        prime_group(0)
        prime_group(1)

        # static: slot(A,0) = z_A(0); slot(B,0) = z_B(0) + tail_A(0)
        emit_z(0)
        y0_buf = pyb.tile([G, 1, O], F32, tag="y0buf", name="y0buf")
        emit_slot(1, 0, 0, y0_buf)
        nc.sync.dma_start(out_flat[0:G, 0:O],
                          y0_buf.rearrange("b k o -> b (k o)"))

        def body(iv0, unroll):
            assert unroll == kf
            ybA = pyb.tile([G, kf, O], F32, tag="ybA", name="ybA")
            ybB = pyb.tile([G, kf, O], F32, tag="ybB", name="ybB")
            for k in range(kf):
                emit_slot(0, 1, k, ybB)   # z_A(t) + tail_B(t-1)
                emit_slot(1, 0, k, ybA)   # z_B(t) + tail_A(t)
            nc.sync.dma_start(out_flat[0:G, ds(iv0 * O, kf * O)],
                              ybA.rearrange("b k o -> b (k o)"))
            nc.sync.dma_start(out_flat[G:Bs, ds((iv0 - 1) * O, kf * O)],
                              ybB.rearrange("b k o -> b (k o)"))

        if dynamic:
            tc.For_i_unrolled_general(
                1, nsteps, 1, body, max_unroll=kf,
                hint_engines=(mybir.EngineType.PE, mybir.EngineType.Activation,
                              mybir.EngineType.DVE, mybir.EngineType.SP,
                              mybir.EngineType.Pool))
        else:
            for t0 in range(1, nsteps, kf):
                body(t0, kf)

        # epilogue: tail_B(nsteps-1), no z
        yl_buf = pyb.tile([G, 1, O], F32, tag="ylbuf", name="ylbuf")
        emit_slot(0, 1, 0, yl_buf, do_z=False, last=True)
        nc.sync.dma_start(out_flat[G:Bs, (nsteps - 1) * O:nsteps * O],
                          yl_buf.rearrange("b k o -> b (k o)"))

    nc.compile()
    return nc


def shard_inputs(inputs, nsteps=T):
    maps = []
    for c in range(NCORES):
        m = {}
        sl = slice(c * Bs, (c + 1) * Bs)
        for k, v in inputs.items():
            v = np.asarray(v, dtype=np.float32)
            m[k] = v[sl] if k in ("h", "s0") else v
        maps.append(m)
    return maps


def bias_flags(inputs):
    return {k: bool(np.any(np.asarray(inputs[k]) != 0))
            for k in ("by1", "by2", "by3", "be1", "br", "bu")}


_CACHE = {}


FP8_DEFAULT = False


def _get_nc(bias_key, fp8=None):
    fp8 = FP8_DEFAULT if fp8 is None else fp8
    key = (bias_key, fp8)
    if key not in _CACHE:
        _CACHE[key] = build_nc(
            nsteps=T, kf=5, dynamic=True, fp8=fp8,
            bias_on=dict(zip(("by1", "by2", "by3", "be1", "br", "bu"),
                             bias_key)))
    return _CACHE[key]


def kernel(**inputs) -> np.ndarray:
    from concourse.bass_utils import run_bass_kernel_spmd

    flags = bias_flags(inputs)
    nc = _get_nc(tuple(flags[k] for k in ("by1", "by2", "by3", "be1", "br", "bu")))
    in_maps = shard_inputs(inputs)
    res = run_bass_kernel_spmd(nc, in_maps, list(range(NCORES)))
    out = np.concatenate([res.results[c]["out"] for c in range(NCORES)], axis=0)
    return out.astype(np.float32)
